# revision 2
# baseline (speedup 1.0000x reference)
"""Trainium2 Bass kernel for elementwise i1e(z) = exp(-|z|) * I1(z),
z in [0.1, 50], shape (32, 4096, 1024) f32, data-parallel over 8 cores.

v2: minimal-pass algorithm (memory-regime).
    a = ln(z + B)              (ACT, 1 pass)
    w = exp(-0.5 * a)          (ACT, 1 pass)  -> w = rsqrt(z+B)
    u = w*w (in-body), r = ((c6*u + c5)*u + c4)*u + c3      (DVE op1)
    out = (((r*u + c2)*u + c1)*u + c0) * w                  (DVE op2)

Max relative error vs scipy.special.i1e: ~6.6e-4 (deg-6 minimax fit).
2 DMA + 2 ACT + 2 DVE per tile; DMA-bound at ~375 us/core roofline.
"""
import numpy as np

NCORES = 8
NT, P, FD = 32, 128, 4096          # per-core: 32 tiles of [128, 4096] f32
FULL_SHAPE = (32, 4096, 1024)
PER_CORE = (4, 4096, 1024)

_B = 2.0
# deg-6 minimax-relative fit of i1e(x)*sqrt(x+B) in u = 1/(x+B), x in [.1,50]
_C = [0.39664357900619507, 0.41442710161209106, -3.587209463119507,
      34.073707580566406, -144.4710693359375, 219.4018096923828,
      -114.7291488647461]

_state = {}


def _register_ops():
    import concourse.dve_ops as dve_ops
    from concourse.dve_spec import (
        Spec, Src0, Src1, C0, C1, C2, C3, _spill_c3_to_src1, lower,
        _has_src1, sq,
    )
    from concourse.dve_uop import DveOpSpec

    if "IVE2_P1" in dve_ops._SUB_OPCODE_FOR_NAME:
        return {o.name: o for o in dve_ops.OPS}

    f32 = np.float32

    def ref_p1(in0, in1, s0, s1, imm2):
        c3 = np.asarray(in1, f32).reshape(-1, 1)
        w = in0.astype(f32)
        u = w * w
        return (((s0 * u + s1) * u + imm2) * u + c3).astype(f32)

    def ref_p2(in0, in1, s0, s1, imm2):
        r, w = in0.astype(f32), in1.astype(f32)
        u = w * w
        return ((((r * u + s0) * u + s1) * u + imm2) * w).astype(f32)

    u0 = sq(Src0)
    u1 = sq(Src1)
    specs = [
        # r = ((C0*u + C1)*u + C2)*u + C3, u = Src0^2   (C3 spills to src1)
        ("IVE2_P1", Spec(
            body=_spill_c3_to_src1(((C0 * u0 + C1) * u0 + C2) * u0 + C3),
            reference=ref_p1)),
        # out = (((Src0*u + C0)*u + C1)*u + C2) * Src1, u = Src1^2
        ("IVE2_P2", Spec(
            body=(((Src0 * u1 + C0) * u1 + C1) * u1 + C2) * Src1,
            reference=ref_p2)),
    ]
    new_ops = []
    for name, spec in specs:
        op = dve_ops.DveOp(name, spec, subdim=False, uops_sha={})
        dve_ops.OPS.append(op)
        new_ops.append(op)
    dve_ops._SUB_OPCODE_FOR_NAME.update(
        {op.name: dve_ops._CUSTOM_DVE_ROW_BASE + i
         for i, op in enumerate(dve_ops.OPS)}
    )
    dve_ops.CUSTOM_DVE_SPECS.update({op.name: op.spec for op in new_ops})
    for op in new_ops:
        shas = {}
        for ver in ("v3", "v4"):
            try:
                s = DveOpSpec(
                    name=op.name,
                    opcode=dve_ops.get_dve_sub_opcode(op.name),
                    uops=lower(op.spec, ver=ver),
                    rd1_en=_has_src1(op.spec),
                )
                shas[ver] = s.sha(ver)
            except Exception:
                pass
        object.__setattr__(op, "uops_sha", shas)
    return {o.name: o for o in dve_ops.OPS}


def _build_nc(reps: int = 1):
    import concourse.bacc as bacc
    import concourse.tile as tile
    from concourse import mybir
    from contextlib import ExitStack

    ops = _register_ops()
    c = [float(np.float32(q)) for q in _C]
    F32 = mybir.dt.float32
    AF = mybir.ActivationFunctionType
    P1, P2 = ops["IVE2_P1"], ops["IVE2_P2"]

    nc = bacc.Bacc(
        "TRN2", target_bir_lowering=False, debug=False,
        enable_asserts=True, num_devices=NCORES,
    )
    z = nc.dram_tensor("z", [NT, P, FD], F32, kind="ExternalInput").ap()
    out = nc.dram_tensor("out", [NT, P, FD], F32, kind="ExternalOutput").ap()

    with tile.TileContext(nc) as tc, ExitStack() as ctx:
        cpool = ctx.enter_context(tc.tile_pool(name="const", bufs=1))
        bias_b = cpool.tile([P, 1], F32, tag="bias_b")
        nc.vector.memset(bias_b[:], _B)
        bias_0 = cpool.tile([P, 1], F32, tag="bias_0")
        nc.vector.memset(bias_0[:], 0.0)
        c3t = cpool.tile([P, 1], F32, tag="c3t")
        nc.vector.memset(c3t[:], c[3])

        pools = {}
        for name, bufs in [("x", 2), ("w", 2), ("t", 3), ("o", 2)]:
            pools[name] = ctx.enter_context(tc.tile_pool(name=name, bufs=bufs))

        for _ in range(reps):
            for i in range(NT):
                xt = pools["x"].tile([P, FD], F32, tag="x")
                nc.sync.dma_start(out=xt[:], in_=z[i])
                at = pools["t"].tile([P, FD], F32, tag="a")
                nc.scalar.activation(at[:], xt[:], AF.Ln, bias=bias_b[:],
                                     scale=1.0)
                wt = pools["w"].tile([P, FD], F32, tag="w")
                nc.scalar.activation(wt[:], at[:], AF.Exp, bias=bias_0[:],
                                     scale=-0.5)
                rt = pools["t"].tile([P, FD], F32, tag="r")
                nc.vector._custom_dve(P1, out=rt[:], in0=wt[:], in1=c3t[:],
                                      s0=c[6], s1=c[5], imm2=c[4])
                ot = pools["o"].tile([P, FD], F32, tag="o")
                nc.vector._custom_dve(P2, out=ot[:], in0=rt[:], in1=wt[:],
                                      s0=c[2], s1=c[1], imm2=c[0])
                nc.sync.dma_start(out=out[i], in_=ot[:])
    nc.compile()
    return nc


def _get_nc():
    if "nc" not in _state:
        _state["nc"] = _build_nc()
    return _state["nc"]


def kernel(z: np.ndarray) -> np.ndarray:
    from concourse.bass_utils import run_bass_kernel_spmd

    z = np.ascontiguousarray(z, dtype=np.float32)
    assert z.shape == FULL_SHAPE, z.shape
    nc = _get_nc()
    shards = z.reshape(NCORES, NT, P, FD)
    in_maps = [{"z": shards[i]} for i in range(NCORES)]
    try:
        res = run_bass_kernel_spmd(nc, in_maps, list(range(NCORES)))
    except Exception:
        res = run_bass_kernel_spmd(nc, in_maps, list(range(NCORES)))
    outs = [res.results[i]["out"].reshape(PER_CORE) for i in range(NCORES)]
    return np.concatenate(outs, axis=0)


# revision 3
# speedup vs baseline: 1.4538x; 1.4538x over previous
"""Trainium2 Bass kernel for elementwise i1e(z) = exp(-|z|) * I1(z),
z in [0.1, 50], shape (32, 4096, 1024) f32, data-parallel over 8 cores.

Algorithm (minimal-pass, memory-regime):
    w = |z + B|^(-1/2)          (ACT Abs_reciprocal_sqrt, 1 pass)
    u = w*w (in-DVE-body), r = ((c6*u + c5)*u + c4)*u + c3   (DVE op1)
    out = (((r*u + c2)*u + c1)*u + c0) * w                   (DVE op2)

deg-6 minimax fit of i1e(x)*sqrt(x+B) in u = 1/(x+B); max rel error
vs scipy.special.i1e ~8.6e-4 on hardware. Per tile: 1 DMA-in (sync
HWDGE queue), 1 ACT, 2 DVE, 1 DMA-out (scalar-engine HWDGE queue --
separate queue so output stores don't head-of-line-block input loads).
HBM-bound: ~134 MB/core/pass with all 8 cores saturating chip HBM.
"""
import numpy as np

NCORES = 8
NT, P, FD = 32, 128, 4096          # per-core: 32 tiles of [128, 4096] f32
FULL_SHAPE = (32, 4096, 1024)
PER_CORE = (4, 4096, 1024)

_B = 2.0
# deg-6 minimax-relative fit of i1e(x)*sqrt(x+B) in u = 1/(x+B), x in [.1,50]
_C = [0.39664357900619507, 0.41442710161209106, -3.587209463119507,
      34.073707580566406, -144.4710693359375, 219.4018096923828,
      -114.7291488647461]

_state = {}


def _register_ops():
    import concourse.dve_ops as dve_ops
    from concourse.dve_spec import (
        Spec, Src0, Src1, C0, C1, C2, C3, _spill_c3_to_src1, lower,
        _has_src1, sq,
    )
    from concourse.dve_uop import DveOpSpec

    if "IVE2_P1" in dve_ops._SUB_OPCODE_FOR_NAME:
        return {o.name: o for o in dve_ops.OPS}

    f32 = np.float32

    def ref_p1(in0, in1, s0, s1, imm2):
        c3 = np.asarray(in1, f32).reshape(-1, 1)
        w = in0.astype(f32)
        u = w * w
        return (((s0 * u + s1) * u + imm2) * u + c3).astype(f32)

    def ref_p2(in0, in1, s0, s1, imm2):
        r, w = in0.astype(f32), in1.astype(f32)
        u = w * w
        return ((((r * u + s0) * u + s1) * u + imm2) * w).astype(f32)

    u0 = sq(Src0)
    u1 = sq(Src1)
    specs = [
        # r = ((C0*u + C1)*u + C2)*u + C3, u = Src0^2   (C3 spills to src1)
        ("IVE2_P1", Spec(
            body=_spill_c3_to_src1(((C0 * u0 + C1) * u0 + C2) * u0 + C3),
            reference=ref_p1)),
        # out = (((Src0*u + C0)*u + C1)*u + C2) * Src1, u = Src1^2
        ("IVE2_P2", Spec(
            body=(((Src0 * u1 + C0) * u1 + C1) * u1 + C2) * Src1,
            reference=ref_p2)),
    ]
    new_ops = []
    for name, spec in specs:
        op = dve_ops.DveOp(name, spec, subdim=False, uops_sha={})
        dve_ops.OPS.append(op)
        new_ops.append(op)
    dve_ops._SUB_OPCODE_FOR_NAME.update(
        {op.name: dve_ops._CUSTOM_DVE_ROW_BASE + i
         for i, op in enumerate(dve_ops.OPS)}
    )
    dve_ops.CUSTOM_DVE_SPECS.update({op.name: op.spec for op in new_ops})
    for op in new_ops:
        shas = {}
        for ver in ("v3", "v4"):
            try:
                s = DveOpSpec(
                    name=op.name,
                    opcode=dve_ops.get_dve_sub_opcode(op.name),
                    uops=lower(op.spec, ver=ver),
                    rd1_en=_has_src1(op.spec),
                )
                shas[ver] = s.sha(ver)
            except Exception:
                pass
        object.__setattr__(op, "uops_sha", shas)
    return {o.name: o for o in dve_ops.OPS}


def _emit_pass(nc, tc, ctx, ops, c, z, out, mybir):
    F32 = mybir.dt.float32
    AF = mybir.ActivationFunctionType
    P1, P2 = ops["IVE2_P1"], ops["IVE2_P2"]

    cpool = ctx.enter_context(tc.tile_pool(name="const", bufs=1))
    bias_b = cpool.tile([P, 1], F32, tag="bias_b")
    nc.vector.memset(bias_b[:], _B)
    c3t = cpool.tile([P, 1], F32, tag="c3t")
    nc.vector.memset(c3t[:], c[3])

    pools = {}
    for name, bufs in [("x", 3), ("w", 2), ("t", 2), ("o", 2)]:
        pools[name] = ctx.enter_context(tc.tile_pool(name=name, bufs=bufs))

    def one_pass():
        for i in range(NT):
            xt = pools["x"].tile([P, FD], F32, tag="x")
            nc.sync.dma_start(out=xt[:], in_=z[i])
            wt = pools["w"].tile([P, FD], F32, tag="w")
            nc.scalar.activation(wt[:], xt[:], AF.Abs_reciprocal_sqrt,
                                 bias=bias_b[:], scale=1.0)
            rt = pools["t"].tile([P, FD], F32, tag="r")
            nc.vector._custom_dve(P1, out=rt[:], in0=wt[:], in1=c3t[:],
                                  s0=c[6], s1=c[5], imm2=c[4])
            ot = pools["o"].tile([P, FD], F32, tag="o")
            nc.vector._custom_dve(P2, out=ot[:], in0=rt[:], in1=wt[:],
                                  s0=c[2], s1=c[1], imm2=c[0])
            nc.scalar.dma_start(out=out[i], in_=ot[:])

    return one_pass


def _build_nc(reps: int = 1):
    import concourse.bacc as bacc
    import concourse.tile as tile
    from concourse import mybir
    from contextlib import ExitStack

    ops = _register_ops()
    c = [float(np.float32(q)) for q in _C]
    F32 = mybir.dt.float32

    nc = bacc.Bacc(
        "TRN2", target_bir_lowering=False, debug=False,
        enable_asserts=True, num_devices=NCORES,
    )
    z = nc.dram_tensor("z", [NT, P, FD], F32, kind="ExternalInput").ap()
    out = nc.dram_tensor("out", [NT, P, FD], F32, kind="ExternalOutput").ap()

    with tile.TileContext(nc) as tc, ExitStack() as ctx:
        one_pass = _emit_pass(nc, tc, ctx, ops, c, z, out, mybir)
        for _ in range(reps):
            one_pass()
    nc.compile()
    return nc


def _build_nc_loop():
    """Timing-only build: extra uint32 [1,1] input `reps_in` drives a
    hardware For_i loop around the pass, so one NEFF times any rep count.
    Not used by kernel()."""
    import concourse.bacc as bacc
    import concourse.tile as tile
    from concourse import mybir
    from contextlib import ExitStack

    ops = _register_ops()
    c = [float(np.float32(q)) for q in _C]
    U32 = mybir.dt.uint32
    F32 = mybir.dt.float32

    nc = bacc.Bacc(
        "TRN2", target_bir_lowering=False, debug=False,
        enable_asserts=True, num_devices=NCORES,
    )
    z = nc.dram_tensor("z", [NT, P, FD], F32, kind="ExternalInput").ap()
    reps_in = nc.dram_tensor("reps_in", [1, 1], U32,
                             kind="ExternalInput").ap()
    out = nc.dram_tensor("out", [NT, P, FD], F32, kind="ExternalOutput").ap()

    with tile.TileContext(nc) as tc, ExitStack() as ctx:
        rpool = ctx.enter_context(tc.tile_pool(name="reps", bufs=1))
        rt_tile = rpool.tile([1, 1], U32, tag="reps")
        nc.sync.dma_start(out=rt_tile[:], in_=reps_in)
        n_reps = nc.values_load(rt_tile[:], min_val=0, max_val=100000,
                                skip_runtime_bounds_check=True)
        one_pass = _emit_pass(nc, tc, ctx, ops, c, z, out, mybir)
        with tc.For_i(0, n_reps, 1, name="reploop"):
            one_pass()
    nc.compile()
    return nc


def _get_nc():
    if "nc" not in _state:
        _state["nc"] = _build_nc()
    return _state["nc"]


def kernel(z: np.ndarray) -> np.ndarray:
    from concourse.bass_utils import run_bass_kernel_spmd

    z = np.ascontiguousarray(z, dtype=np.float32)
    assert z.shape == FULL_SHAPE, z.shape
    nc = _get_nc()
    shards = z.reshape(NCORES, NT, P, FD)
    in_maps = [{"z": shards[i]} for i in range(NCORES)]
    try:
        res = run_bass_kernel_spmd(nc, in_maps, list(range(NCORES)))
    except Exception:
        res = run_bass_kernel_spmd(nc, in_maps, list(range(NCORES)))
    outs = [res.results[i]["out"].reshape(PER_CORE) for i in range(NCORES)]
    return np.concatenate(outs, axis=0)
